# revision 4
# baseline (speedup 1.0000x reference)
"""Trainium2 Bass kernel for nn_BaseCPNN (vq_codebook).

reference math:
    d2[b,h]  = ||x_b||^2 + ||w_h||^2 - 2 x_b.w_h      (kohonen distances)
    winners  = argmin_h d2                            (first index on ties)
    output   = grossberg_weights.T[winners]           (pure row gather)

Device strategy (8 NeuronCores, SPMD):
  - Shard the codebook (HID=16384) across cores: 2048 codewords per core.
  - argmin_h d2 == argmax_h (x.w_h - ||w_h||^2/2): x2 is row-constant.
  - Dot products at full PE rate via a 3-term hi/lo split:
        x.w ~= xh.wh + xh.wl + xl.wh   (hi/lo fp16 or bf16 pairs)
    Max dot error ~3e-5 (bf16) while the data's min winner gap is 1.55e-4,
    so winners are exact => output is bit-exact (it is a pure gather).
  - Per-core top-1 via DVE max/max_index over score tiles.
  - Global argmin: AllReduce-max of the per-core best scores, then a
    masked ReduceScatter-min of the candidate indices (preserves the
    reference's first-index tie-breaking).
  - Each core gathers grossberg rows for its 512-row batch slice via
    indirect DMA and writes its slice of the output.
"""

import os
import sys

sys.path.insert(0, "/opt/trn_rl_repo")

import numpy as np

N_CORES = 8
B, IN, HID, OUT = 4096, 512, 16384, 1000
HC = HID // N_CORES          # 2048 codewords per core
BC = B // N_CORES            # 512 batch rows gathered per core
KC = IN // 128               # 4 contraction chunks
M_TILES = B // 128           # 32
N_TILES = HC // 512          # 4
MT_PER_CORE = BC // 128      # 4 output row-tiles per core
BIG = 1.0e9                  # > any valid index, for the masked min

# lo/hi split dtype: fp16 keeps ~22 mantissa bits (margin ~200x),
# bf16 keeps ~16 (margin ~5x on this data).
SPLIT_DT = os.environ.get("CPNN_SPLIT_DT", "float16")

_compiled = None


def _build():
    from concourse import bacc, bass, mybir
    from concourse.tile import TileContext

    f32 = mybir.dt.float32
    i32 = mybir.dt.int32
    u32 = mybir.dt.uint32
    f16 = getattr(mybir.dt, SPLIT_DT)

    nc = bacc.Bacc(num_devices=N_CORES)

    xh_in = nc.declare_dram_parameter("xh", [IN, B], f16, isOutput=False)
    xl_in = nc.declare_dram_parameter("xl", [IN, B], f16, isOutput=False)
    kh_in = nc.declare_dram_parameter("kh", [IN, HC], f16, isOutput=False)
    kl_in = nc.declare_dram_parameter("kl", [IN, HC], f16, isOutput=False)
    gwt_in = nc.declare_dram_parameter("gwt", [HID, OUT], f32, isOutput=False)
    hoff_in = nc.declare_dram_parameter("hoff", [128, 1], f32, isOutput=False)

    y_out = nc.declare_dram_parameter("y", [BC, OUT], f32, isOutput=True)
    win_out = nc.declare_dram_parameter("winners", [BC], i32, isOutput=True)

    # internal DRAM for the collectives
    sc_in = nc.dram_tensor("sc_in", [B], f32)
    sc_ar = nc.dram_tensor("sc_ar", [B], f32, addr_space="Shared")
    mi_in = nc.dram_tensor("mi_in", [B], f32)
    mi_rs = nc.dram_tensor("mi_rs", [BC], f32)

    with TileContext(nc) as tc:
        with (
            tc.tile_pool(name="kw", bufs=1) as kw_pool,
            tc.tile_pool(name="const", bufs=1) as const_pool,
            tc.tile_pool(name="xmt", bufs=3) as x_pool,
            tc.tile_pool(name="score", bufs=3) as score_pool,
            tc.tile_pool(name="small", bufs=3) as small_pool,
            tc.tile_pool(name="acc", bufs=1) as acc_pool,
            tc.tile_pool(name="gat", bufs=2) as gat_pool,
            tc.tile_pool(name="ps", bufs=2, space="PSUM") as ps_pool,
        ):
            # ---- resident codebook chunk (hi/lo), [K=128, HC] per k-chunk
            kh_t = [
                kw_pool.tile([128, HC], f16, tag=f"kh{k}", name=f"kh{k}")
                for k in range(KC)
            ]
            kl_t = [
                kw_pool.tile([128, HC], f16, tag=f"kl{k}", name=f"kl{k}")
                for k in range(KC)
            ]
            for k in range(KC):
                nc.sync.dma_start(out=kh_t[k][:], in_=kh_in[k * 128:(k + 1) * 128, :])
                nc.sync.dma_start(out=kl_t[k][:], in_=kl_in[k * 128:(k + 1) * 128, :])

            # ---- w2b[p, h] = sum_k (kh+kl)^2 (exact fp32), broadcast over p,
            # then scaled by -1/2: score = dot - w2/2 lands in one DVE subtract.
            ones_t = const_pool.tile([128, 128], f32, tag="ones")
            nc.vector.memset(ones_t[:], 1.0)
            ps_w2 = ps_pool.tile([128, HC], f32, tag="ps")
            for k in range(KC):
                wsum = score_pool.tile([128, HC], f32, tag="score")
                nc.vector.tensor_add(out=wsum[:], in0=kh_t[k][:], in1=kl_t[k][:])
                nc.vector.tensor_mul(out=wsum[:], in0=wsum[:], in1=wsum[:])
                for ns in range(N_TILES):
                    sl = slice(ns * 512, (ns + 1) * 512)
                    nc.tensor.matmul(
                        out=ps_w2[:, sl], lhsT=ones_t[:], rhs=wsum[:, sl],
                        start=(k == 0), stop=(k == KC - 1),
                    )
            w2b = const_pool.tile([128, HC], f32, tag="w2b")
            nc.scalar.activation(
                out=w2b[:], in_=ps_w2[:],
                func=mybir.ActivationFunctionType.Copy, scale=-0.5,
            )

            hoff_t = const_pool.tile([128, 1], f32, tag="hoff")
            nc.sync.dma_start(out=hoff_t[:], in_=hoff_in[:])

            # ---- per-core best score / best index accumulators
            best_sb = acc_pool.tile([128, M_TILES], f32, tag="best")
            bidx_sb = acc_pool.tile([128, M_TILES], f32, tag="bidx")

            # ---- main loop over the 32 batch M-tiles
            for m in range(M_TILES):
                # x M-tile, hi/lo: [p=K-part, kc*128 + mcol]
                xh_mt = x_pool.tile([128, KC * 128], f16, tag="xh")
                xl_mt = x_pool.tile([128, KC * 128], f16, tag="xl")
                src_h = xh_in[:].rearrange("(a p) b -> p a b", a=KC)[
                    :, :, m * 128:(m + 1) * 128
                ]
                src_l = xl_in[:].rearrange("(a p) b -> p a b", a=KC)[
                    :, :, m * 128:(m + 1) * 128
                ]
                dst_h = xh_mt[:].rearrange("p (a b) -> p a b", a=KC)
                dst_l = xl_mt[:].rearrange("p (a b) -> p a b", a=KC)
                nc.sync.dma_start(out=dst_h, in_=src_h)
                nc.sync.dma_start(out=dst_l, in_=src_l)

                ps = ps_pool.tile([128, HC], f32, tag="ps")
                terms = [(xh_mt, kh_t), (xh_mt, kl_t), (xl_mt, kh_t)]
                n_acc = len(terms) * KC
                ti = 0
                for x_t, kw_list in terms:
                    for k in range(KC):
                        lhsT = x_t[:, k * 128:(k + 1) * 128]
                        for ns in range(N_TILES):
                            sl = slice(ns * 512, (ns + 1) * 512)
                            nc.tensor.matmul(
                                out=ps[:, sl], lhsT=lhsT, rhs=kw_list[k][:, sl],
                                start=(ti == 0), stop=(ti == n_acc - 1),
                            )
                        ti += 1

                score = score_pool.tile([128, HC], f32, tag="score")
                nc.vector.tensor_add(out=score[:], in0=ps[:], in1=w2b[:])

                mx = small_pool.tile([128, 8], f32, tag="mx")
                mi = small_pool.tile([128, 8], u32, tag="mi")
                nc.vector.max(out=mx[:], in_=score[:])
                nc.vector.max_index(mi[:], mx[:], score[:])
                nc.vector.tensor_copy(out=best_sb[:, m:m + 1], in_=mx[:, 0:1])
                nc.vector.tensor_copy(out=bidx_sb[:, m:m + 1], in_=mi[:, 0:1])

            # local chunk index -> global codeword index
            nc.vector.tensor_scalar_add(bidx_sb[:], bidx_sb[:], hoff_t[:])

            # ---- global argmin across cores
            nc.sync.dma_start(
                out=sc_in[:].rearrange("(m p) -> p m", p=128), in_=best_sb[:]
            )
            nc.gpsimd.collective_compute(
                "AllReduce", mybir.AluOpType.max,
                replica_groups=[list(range(N_CORES))],
                ins=[sc_in[:]], outs=[sc_ar[:]],
            )
            g_sb = acc_pool.tile([128, M_TILES], f32, tag="gmax")
            nc.sync.dma_start(
                out=g_sb[:], in_=sc_ar[:].rearrange("(m p) -> p m", p=128)
            )
            mask = acc_pool.tile([128, M_TILES], f32, tag="mask")
            nc.vector.tensor_tensor(
                out=mask[:], in0=best_sb[:], in1=g_sb[:],
                op=mybir.AluOpType.is_equal,
            )
            # masked = mask * bidx + (1-mask) * BIG
            nc.vector.tensor_mul(out=bidx_sb[:], in0=bidx_sb[:], in1=mask[:])
            nc.vector.tensor_scalar(
                mask[:], mask[:], -BIG, scalar2=BIG,
                op0=mybir.AluOpType.mult, op1=mybir.AluOpType.add,
            )
            nc.vector.tensor_add(out=bidx_sb[:], in0=bidx_sb[:], in1=mask[:])
            nc.sync.dma_start(
                out=mi_in[:].rearrange("(m p) -> p m", p=128), in_=bidx_sb[:]
            )
            nc.gpsimd.collective_compute(
                "ReduceScatter", mybir.AluOpType.min,
                replica_groups=[list(range(N_CORES))],
                ins=[mi_in[:]], outs=[mi_rs[:]],
            )

            # ---- winners for this core's batch slice + gather
            win_f = acc_pool.tile([128, MT_PER_CORE], f32, tag="winf")
            nc.sync.dma_start(
                out=win_f[:], in_=mi_rs[:].rearrange("(m p) -> p m", p=128)
            )
            win_i = acc_pool.tile([128, MT_PER_CORE], i32, tag="wini")
            nc.vector.tensor_copy(out=win_i[:], in_=win_f[:])
            nc.sync.dma_start(
                out=win_out[:].rearrange("(m p) -> p m", p=128), in_=win_i[:]
            )
            for j in range(MT_PER_CORE):
                g_tile = gat_pool.tile([128, OUT], f32, tag="gt")
                nc.gpsimd.indirect_dma_start(
                    out=g_tile[:], out_offset=None,
                    in_=gwt_in[:],
                    in_offset=bass.IndirectOffsetOnAxis(
                        ap=win_i[:, j:j + 1], axis=0
                    ),
                )
                nc.sync.dma_start(
                    out=y_out[j * 128:(j + 1) * 128, :], in_=g_tile[:]
                )

    nc.compile()
    return nc


def _get_nc():
    global _compiled
    if _compiled is None:
        _compiled = _build()
    return _compiled


def kernel(x, kohonen_weights, grossberg_weights, _trace=False):
    from concourse.bass_utils import run_bass_kernel_spmd

    nc = _get_nc()
    f16 = np.dtype(SPLIT_DT if SPLIT_DT == "float16" else "float32")
    if SPLIT_DT == "bfloat16":
        import ml_dtypes
        f16 = np.dtype(ml_dtypes.bfloat16)

    x_t = np.ascontiguousarray(np.asarray(x, np.float32).T)          # [IN, B]
    xh = x_t.astype(f16)
    xl = (x_t - xh.astype(np.float32)).astype(f16)
    kw_t = np.asarray(kohonen_weights, np.float32).T                  # [IN, HID] view
    gw_t = np.ascontiguousarray(np.asarray(grossberg_weights, np.float32).T)

    in_maps = []
    for i in range(N_CORES):
        kwc = np.ascontiguousarray(kw_t[:, i * HC:(i + 1) * HC])
        kh = kwc.astype(f16)
        kl = (kwc - kh.astype(np.float32)).astype(f16)
        in_maps.append({
            "xh": xh, "xl": xl, "kh": kh, "kl": kl, "gwt": gw_t,
            "hoff": np.full([128, 1], float(i * HC), np.float32),
        })

    res = run_bass_kernel_spmd(
        nc, in_maps, list(range(N_CORES)), trace=_trace
    )
    y = np.concatenate([res.results[i]["y"] for i in range(N_CORES)], axis=0)
    winners = np.concatenate(
        [res.results[i]["winners"] for i in range(N_CORES)], axis=0
    ).astype(np.int32)
    if _trace:
        kernel._last_result = res
    return y, winners
